# revision 6
# baseline (speedup 1.0000x reference)
"""Trainium2 kernel for nn_CoarsenBlock (topk graph coarsening).

Strategy
--------
Per-graph math is independent -> data-parallel over B=8 graphs, one graph
per NeuronCore (8 cores).

The reference output S = norm_adj * cut_alpha has only K=205 nonzero
columns (the top-k attention nodes; verified: exactly K positive entries of
cut_alpha per graph, with >=1.6e-4 margin at the cut vs <=4e-7 float noise).
So the heavy O(N^3) dense einsums collapse to gathered forms:

    Sg   = S[:, topi]                  [N, K]   (host, cheap O(N*K))
    Tg   = adj @ Sg                    [N, K]   (device matmul, dominant)
    Cg   = Sg^T @ Tg                   [K, K]   (device matmul)
    x_cg = Sg^T @ x                    [K, F]   (device matmul)

The device streams adj (16.8 MB/graph) once from HBM - that stream is the
roofline for this problem - while PE does the three matmuls in fp32r
(single-pass fp32 PE mode, 1 cyc/row at moving free dim >= 256). Cheap
O(N^2) host prep (row sums, one matvec for the GCN attention score, top-k)
and the scatter of the K nonzero columns/rows into the full-size outputs
are done host-side in numpy.

kernel(**inputs) takes FULL inputs and returns the FULL output tuple
(x_c, coarse, S, topi) exactly like the reference.
"""

import numpy as np

B, N, FDIM = 8, 2048, 256
K = 205          # int(N * 0.1) + 1
KP = 256         # K padded (zero cols); 256 unlocks the fp32r 1-cyc/row PE path
P = 128          # SBUF partitions
NT = N // P      # 16 row tiles
SW = 512         # adj strip width (columns per DMA)
NS = N // SW     # 4 strips

_CACHE = {}


def _build_program():
    import concourse.tile as tile
    from concourse import bacc, mybir

    nc = bacc.Bacc("TRN2", target_bir_lowering=False, debug=False, num_devices=B)

    f32 = mybir.dt.float32
    f32r = mybir.dt.float32r  # same bits as f32; fast single-pass PE mode

    adj = nc.dram_tensor("adj", [N, N], f32r, kind="ExternalInput")
    sg = nc.dram_tensor("sg", [N, KP], f32r, kind="ExternalInput")
    x = nc.dram_tensor("x", [N, FDIM], f32r, kind="ExternalInput")
    cg = nc.dram_tensor("cg", [KP, KP], f32, kind="ExternalOutput")
    xc = nc.dram_tensor("xc", [KP, FDIM], f32, kind="ExternalOutput")

    with tile.TileContext(nc) as tc:
        with (
            tc.tile_pool(name="resident", bufs=1) as resident,
            tc.tile_pool(name="strips", bufs=3) as strips,
            tc.tile_pool(name="tgp", bufs=4) as tgp,
            tc.tile_pool(name="outp", bufs=1) as outp,
            tc.tile_pool(name="ps_t", bufs=2, space="PSUM") as ps_t,
            tc.tile_pool(name="ps_acc", bufs=1, space="PSUM") as ps_acc,
        ):
            # Resident operands: Sg [128, 16, 256] and x [128, 16, 256],
            # partition p holds row i*128+p of the logical [2048, *] matrix.
            # Loaded in halves on the scalar HWDGE ring so they stream in
            # parallel with the adj strips on the sync ring.
            sg_r = sg.ap().rearrange("(i p) k -> p i k", p=P)
            sg_sb = resident.tile([P, NT, KP], f32r)
            H = NT // 2
            nc.scalar.dma_start(out=sg_sb[:, 0:H, :], in_=sg_r[:, 0:H, :])
            nc.scalar.dma_start(out=sg_sb[:, H:NT, :], in_=sg_r[:, H:NT, :])
            x_r = x.ap().rearrange("(i p) f -> p i f", p=P)
            x_sb = resident.tile([P, NT, FDIM], f32r)
            nc.scalar.dma_start(out=x_sb[:, 0:H, :], in_=x_r[:, 0:H, :])
            nc.scalar.dma_start(out=x_sb[:, H:NT, :], in_=x_r[:, H:NT, :])

            # PSUM accumulators held across the whole n loop.
            cg_ps0 = ps_acc.tile([P, KP], f32)
            cg_ps1 = ps_acc.tile([KP - P, KP], f32)
            xc_ps0 = ps_acc.tile([P, FDIM], f32)
            xc_ps1 = ps_acc.tile([KP - P, FDIM], f32)

            adj_r = adj.ap().rearrange("(i p) m -> p i m", p=P)  # [128, 16, 2048]

            for s in range(NS):
                # 512-col strip of adj (2 KB contiguous per row -> efficient
                # DMA): lhsT blocks for 4 consecutive n output blocks. Two
                # half-strip DMAs so matmuls on the low i blocks can start
                # before the whole strip lands (Tile subtile deps).
                strip = strips.tile([P, NT, SW], f32r, tag="strip")
                nc.sync.dma_start(
                    out=strip[:, 0:H, :],
                    in_=adj_r[:, 0:H, s * SW : (s + 1) * SW],
                )
                nc.sync.dma_start(
                    out=strip[:, H:NT, :],
                    in_=adj_r[:, H:NT, s * SW : (s + 1) * SW],
                )
                for j in range(SW // P):
                    n = s * (SW // P) + j
                    # Tg[n-block] = sum_i adj[i-block, n-block].T @ Sg_i
                    # (adj is exactly symmetric, so adj[i,n] blocks are lhsT).
                    tg_ps = ps_t.tile([P, KP], f32, tag="tg_ps")
                    for i in range(NT):
                        nc.tensor.matmul(
                            tg_ps,
                            lhsT=strip[:, i, j * P : (j + 1) * P],
                            rhs=sg_sb[:, i, :],
                            start=(i == 0),
                            stop=(i == NT - 1),
                        )
                    tg_sb = tgp.tile([P, KP], f32r, tag="tg_sb")
                    nc.vector.tensor_copy(out=tg_sb, in_=tg_ps)

                    # Cg += Sg_n^T @ Tg_n ; x_cg += Sg_n^T @ x_n
                    nc.tensor.matmul(
                        cg_ps0, lhsT=sg_sb[:, n, 0:P], rhs=tg_sb,
                        start=(n == 0), stop=(n == NT - 1),
                    )
                    nc.tensor.matmul(
                        cg_ps1, lhsT=sg_sb[:, n, P:KP], rhs=tg_sb,
                        start=(n == 0), stop=(n == NT - 1),
                    )
                    nc.tensor.matmul(
                        xc_ps0, lhsT=sg_sb[:, n, 0:P], rhs=x_sb[:, n, :],
                        start=(n == 0), stop=(n == NT - 1),
                    )
                    nc.tensor.matmul(
                        xc_ps1, lhsT=sg_sb[:, n, P:KP], rhs=x_sb[:, n, :],
                        start=(n == 0), stop=(n == NT - 1),
                    )

            cg0 = outp.tile([P, KP], f32)
            nc.vector.tensor_copy(out=cg0, in_=cg_ps0)
            nc.sync.dma_start(out=cg.ap()[0:P, :], in_=cg0)
            cg1 = outp.tile([KP - P, KP], f32)
            nc.vector.tensor_copy(out=cg1, in_=cg_ps1)
            nc.sync.dma_start(out=cg.ap()[P:KP, :], in_=cg1)
            xc0 = outp.tile([P, FDIM], f32)
            nc.vector.tensor_copy(out=xc0, in_=xc_ps0)
            nc.sync.dma_start(out=xc.ap()[0:P, :], in_=xc0)
            xc1 = outp.tile([KP - P, FDIM], f32)
            nc.vector.tensor_copy(out=xc1, in_=xc_ps1)
            nc.sync.dma_start(out=xc.ap()[P:KP, :], in_=xc1)

    nc.compile()
    return nc


def _host_prep(x, adj, W, b):
    """O(N^2) prep: attention scores, top-k, and the K nonzero S columns."""
    f1 = np.float32
    xw = (x.reshape(B * N, FDIM) @ W).reshape(B, N).astype(f1)
    rs = adj.sum(axis=2, dtype=np.float32)            # [B, N]
    d = ((rs + f1(1.0)) ** f1(-0.5)).astype(f1)
    maskf = (rs > 0).astype(f1)
    v = (d * xw).astype(f1)
    Av = (np.matmul(adj, v[..., None])[..., 0] + v).astype(f1)
    z = (d * Av + b[0]).astype(f1)
    alpha = (f1(1.0) / (f1(1.0) + np.exp(-(z * z)))).astype(f1)

    order = np.argsort(-alpha, axis=1, kind="stable")
    topi = order[:, :K].astype(np.int32)

    sg_pad = np.zeros((B, N, KP), np.float32)
    dmask = (d * maskf).astype(f1)
    for g in range(B):
        ti = topi[g]
        cut = alpha[g, ti[K - 1]]
        cut_alpha = np.maximum((alpha[g] + f1(1e-7)) - cut, f1(0.0))
        w_vec = (d[g, ti] * cut_alpha[ti]).astype(f1)
        cols = adj[g][:, ti].copy()                   # [N, K] = A_hat columns
        cols[ti, np.arange(K)] += f1(1.0)             # + I on the diagonal
        sun = (dmask[g][:, None] * cols * w_vec[None, :]).astype(f1)
        s = sun.sum(axis=1, dtype=np.float32)
        q = (f1(1.0) / np.maximum(s, f1(1e-12))).astype(f1)
        sg_pad[g, :, :K] = sun * q[:, None]
    return sg_pad, topi


def _run_device(adj, sg_pad, x, trace=False, **kw):
    from concourse.bass_utils import run_bass_kernel_spmd

    if "nc" not in _CACHE:
        _CACHE["nc"] = _build_program()
    nc = _CACHE["nc"]
    in_maps = [
        {
            "adj": np.ascontiguousarray(adj[g]),
            "sg": np.ascontiguousarray(sg_pad[g]),
            "x": np.ascontiguousarray(x[g]),
        }
        for g in range(B)
    ]
    return run_bass_kernel_spmd(nc, in_maps, list(range(B)), trace=trace, **kw)


def kernel(x, adj, W, b):
    x = np.ascontiguousarray(np.asarray(x, np.float32))
    adj = np.ascontiguousarray(np.asarray(adj, np.float32))
    W = np.asarray(W, np.float32)
    b = np.asarray(b, np.float32)

    sg_pad, topi = _host_prep(x, adj, W, b)
    res = _run_device(adj, sg_pad, x).results

    f1 = np.float32
    x_c = np.zeros((B, N, FDIM), np.float32)
    coarse = np.zeros((B, N, N), np.float32)
    S = np.zeros((B, N, N), np.float32)
    for g in range(B):
        ti = topi[g]
        cg_dev = res[g]["cg"][:K, :K]
        xc_dev = res[g]["xc"][:K, :]
        coarse[g][np.ix_(ti, ti)] = np.floor(cg_dev * f1(1e4)) / f1(1e4)
        x_c[g][ti, :] = xc_dev
        S[g][:, ti] = sg_pad[g, :, :K]
    return x_c, coarse, S, topi


# revision 11
# speedup vs baseline: 1.0283x; 1.0283x over previous
"""Trainium2 kernel for nn_CoarsenBlock (topk graph coarsening).

Strategy
--------
Per-graph math is independent -> data-parallel over B=8 graphs, one graph
per NeuronCore (8 cores).

The reference output S = norm_adj * cut_alpha has only K=205 nonzero
columns (the top-k attention nodes; verified: exactly K positive entries of
cut_alpha per graph, with >=1.6e-4 margin at the cut vs <=4e-7 float noise).
So the heavy O(N^3) dense einsums collapse to gathered forms:

    Sg   = S[:, topi]                  [N, K]   (host, cheap O(N*K))
    Tg   = adj @ Sg                    [N, K]   (device matmul, dominant)
    Cg   = Sg^T @ Tg                   [K, K]   (device matmul)
    x_cg = Sg^T @ x                    [K, F]   (device matmul)

The device streams adj (16.8 MB/graph) once from HBM - that stream is the
roofline for this problem - while PE does the three matmuls in fp32r
(single-pass fp32 PE mode, 1 cyc/row at moving free dim >= 256). Cheap
O(N^2) host prep (row sums, one matvec for the GCN attention score, top-k)
and the scatter of the K nonzero columns/rows into the full-size outputs
are done host-side in numpy.

kernel(**inputs) takes FULL inputs and returns the FULL output tuple
(x_c, coarse, S, topi) exactly like the reference.
"""

import numpy as np

B, N, FDIM = 8, 2048, 256
K = 205          # int(N * 0.1) + 1
KP = 256         # K padded (zero cols); 256 unlocks the fp32r 1-cyc/row PE path
P = 128          # SBUF partitions
NT = N // P      # 16 row tiles
SW = 256         # adj strip width (columns per DMA)
NS = N // SW     # 8 strips

_CACHE = {}


def _build_program():
    import concourse.tile as tile
    from concourse import bacc, mybir

    nc = bacc.Bacc("TRN2", target_bir_lowering=False, debug=False, num_devices=B)

    f32 = mybir.dt.float32
    f32r = mybir.dt.float32r  # same bits as f32; fast single-pass PE mode

    # adj arrives host-pre-tiled to the exact SBUF strip layout
    # adj_t[s, p, i, c] = adj[i*128 + p, s*SW + c], so every strip DMA is
    # fully contiguous per partition (NT*SW*4 bytes per descriptor).
    adj = nc.dram_tensor("adj", [NS, P, NT, SW], f32r, kind="ExternalInput")
    sg = nc.dram_tensor("sg", [N, KP], f32r, kind="ExternalInput")
    x = nc.dram_tensor("x", [N, FDIM], f32r, kind="ExternalInput")
    cg = nc.dram_tensor("cg", [KP, KP], f32, kind="ExternalOutput")
    xc = nc.dram_tensor("xc", [KP, FDIM], f32, kind="ExternalOutput")

    with tile.TileContext(nc) as tc:
        with (
            tc.tile_pool(name="resident", bufs=1) as resident,
            tc.tile_pool(name="strips", bufs=4) as strips,
            tc.tile_pool(name="tgp", bufs=4) as tgp,
            tc.tile_pool(name="outp", bufs=1) as outp,
            tc.tile_pool(name="ps_t", bufs=3, space="PSUM") as ps_t,
            tc.tile_pool(name="ps_acc", bufs=1, space="PSUM") as ps_acc,
        ):
            # Resident operands: Sg [128, 16, 256] and x [128, 16, 256],
            # partition p holds row i*128+p of the logical [2048, *] matrix.
            # Loaded in halves on the scalar HWDGE ring so they stream in
            # parallel with the adj strips on the sync ring.
            sg_r = sg.ap().rearrange("(i p) k -> p i k", p=P)
            sg_sb = resident.tile([P, NT, KP], f32r)
            H = NT // 2
            nc.scalar.dma_start(out=sg_sb[:, 0:H, :], in_=sg_r[:, 0:H, :])
            nc.scalar.dma_start(out=sg_sb[:, H:NT, :], in_=sg_r[:, H:NT, :])
            x_r = x.ap().rearrange("(i p) f -> p i f", p=P)
            x_sb = resident.tile([P, NT, FDIM], f32r)
            nc.scalar.dma_start(out=x_sb[:, 0:H, :], in_=x_r[:, 0:H, :])
            nc.scalar.dma_start(out=x_sb[:, H:NT, :], in_=x_r[:, H:NT, :])

            # PSUM accumulators held across the whole n loop.
            cg_ps0 = ps_acc.tile([P, KP], f32)
            cg_ps1 = ps_acc.tile([KP - P, KP], f32)
            xc_ps0 = ps_acc.tile([P, FDIM], f32)
            xc_ps1 = ps_acc.tile([KP - P, FDIM], f32)

            adj_r = adj.ap()  # [NS, 128, 16, SW], already SBUF-layout

            for s in range(NS):
                # SW-col strip of adj; host pre-tiling makes the DMA fully
                # contiguous per partition (16*SW*4 B descriptors).
                strip = strips.tile([P, NT, SW], f32r, tag="strip")
                nc.sync.dma_start(out=strip, in_=adj_r[s])
                for j in range(SW // P):
                    n = s * (SW // P) + j
                    # Tg[n-block] = sum_i adj[i-block, n-block].T @ Sg_i
                    # (adj is exactly symmetric, so adj[i,n] blocks are lhsT).
                    tg_ps = ps_t.tile([P, KP], f32, tag="tg_ps")
                    for i in range(NT):
                        nc.tensor.matmul(
                            tg_ps,
                            lhsT=strip[:, i, j * P : (j + 1) * P],
                            rhs=sg_sb[:, i, :],
                            start=(i == 0),
                            stop=(i == NT - 1),
                        )
                    tg_sb = tgp.tile([P, KP], f32r, tag="tg_sb")
                    nc.vector.tensor_copy(out=tg_sb, in_=tg_ps)

                    # Cg += Sg_n^T @ Tg_n ; x_cg += Sg_n^T @ x_n
                    nc.tensor.matmul(
                        cg_ps0, lhsT=sg_sb[:, n, 0:P], rhs=tg_sb,
                        start=(n == 0), stop=(n == NT - 1),
                    )
                    nc.tensor.matmul(
                        cg_ps1, lhsT=sg_sb[:, n, P:KP], rhs=tg_sb,
                        start=(n == 0), stop=(n == NT - 1),
                    )
                    nc.tensor.matmul(
                        xc_ps0, lhsT=sg_sb[:, n, 0:P], rhs=x_sb[:, n, :],
                        start=(n == 0), stop=(n == NT - 1),
                    )
                    nc.tensor.matmul(
                        xc_ps1, lhsT=sg_sb[:, n, P:KP], rhs=x_sb[:, n, :],
                        start=(n == 0), stop=(n == NT - 1),
                    )

            cg0 = outp.tile([P, KP], f32)
            nc.vector.tensor_copy(out=cg0, in_=cg_ps0)
            nc.sync.dma_start(out=cg.ap()[0:P, :], in_=cg0)
            cg1 = outp.tile([KP - P, KP], f32)
            nc.vector.tensor_copy(out=cg1, in_=cg_ps1)
            nc.sync.dma_start(out=cg.ap()[P:KP, :], in_=cg1)
            xc0 = outp.tile([P, FDIM], f32)
            nc.vector.tensor_copy(out=xc0, in_=xc_ps0)
            nc.sync.dma_start(out=xc.ap()[0:P, :], in_=xc0)
            xc1 = outp.tile([KP - P, FDIM], f32)
            nc.vector.tensor_copy(out=xc1, in_=xc_ps1)
            nc.sync.dma_start(out=xc.ap()[P:KP, :], in_=xc1)

    nc.compile()
    return nc


def _host_prep(x, adj, W, b):
    """O(N^2) prep: attention scores, top-k, and the K nonzero S columns."""
    f1 = np.float32
    xw = (x.reshape(B * N, FDIM) @ W).reshape(B, N).astype(f1)
    rs = adj.sum(axis=2, dtype=np.float32)            # [B, N]
    d = ((rs + f1(1.0)) ** f1(-0.5)).astype(f1)
    maskf = (rs > 0).astype(f1)
    v = (d * xw).astype(f1)
    Av = (np.matmul(adj, v[..., None])[..., 0] + v).astype(f1)
    z = (d * Av + b[0]).astype(f1)
    alpha = (f1(1.0) / (f1(1.0) + np.exp(-(z * z)))).astype(f1)

    order = np.argsort(-alpha, axis=1, kind="stable")
    topi = order[:, :K].astype(np.int32)

    sg_pad = np.zeros((B, N, KP), np.float32)
    dmask = (d * maskf).astype(f1)
    for g in range(B):
        ti = topi[g]
        cut = alpha[g, ti[K - 1]]
        cut_alpha = np.maximum((alpha[g] + f1(1e-7)) - cut, f1(0.0))
        w_vec = (d[g, ti] * cut_alpha[ti]).astype(f1)
        cols = adj[g][:, ti].copy()                   # [N, K] = A_hat columns
        cols[ti, np.arange(K)] += f1(1.0)             # + I on the diagonal
        sun = (dmask[g][:, None] * cols * w_vec[None, :]).astype(f1)
        s = sun.sum(axis=1, dtype=np.float32)
        q = (f1(1.0) / np.maximum(s, f1(1e-12))).astype(f1)
        sg_pad[g, :, :K] = sun * q[:, None]
    return sg_pad, topi


def _run_device(adj, sg_pad, x, trace=False, **kw):
    from concourse.bass_utils import run_bass_kernel_spmd

    if "nc" not in _CACHE:
        _CACHE["nc"] = _build_program()
    nc = _CACHE["nc"]
    in_maps = []
    for g in range(B):
        # adj_t[s, p, i, c] = adj[i*128 + p, s*SW + c]
        adj_t = np.ascontiguousarray(
            adj[g].reshape(NT, P, NS, SW).transpose(2, 1, 0, 3)
        )
        in_maps.append(
            {
                "adj": adj_t,
                "sg": np.ascontiguousarray(sg_pad[g]),
                "x": np.ascontiguousarray(x[g]),
            }
        )
    return run_bass_kernel_spmd(nc, in_maps, list(range(B)), trace=trace, **kw)


def kernel(x, adj, W, b):
    x = np.ascontiguousarray(np.asarray(x, np.float32))
    adj = np.ascontiguousarray(np.asarray(adj, np.float32))
    W = np.asarray(W, np.float32)
    b = np.asarray(b, np.float32)

    sg_pad, topi = _host_prep(x, adj, W, b)
    res = _run_device(adj, sg_pad, x).results

    f1 = np.float32
    x_c = np.zeros((B, N, FDIM), np.float32)
    coarse = np.zeros((B, N, N), np.float32)
    S = np.zeros((B, N, N), np.float32)
    for g in range(B):
        ti = topi[g]
        cg_dev = res[g]["cg"][:K, :K]
        xc_dev = res[g]["xc"][:K, :]
        coarse[g][np.ix_(ti, ti)] = np.floor(cg_dev * f1(1e4)) / f1(1e4)
        x_c[g][ti, :] = xc_dev
        S[g][:, ti] = sg_pad[g, :, :K]
    return x_c, coarse, S, topi


# revision 15
# speedup vs baseline: 1.1702x; 1.1380x over previous
"""Trainium2 kernel for nn_CoarsenBlock (topk graph coarsening).

Strategy
--------
Per-graph math is independent -> data-parallel over B=8 graphs, one graph
per NeuronCore (8 cores).

The reference output S = norm_adj * cut_alpha has only K=205 nonzero
columns (the top-k attention nodes; verified: exactly K positive entries of
cut_alpha per graph, with >=1.6e-4 margin at the cut vs <=4e-7 float noise).
So the heavy O(N^3) dense einsums collapse to gathered forms:

    Sg   = S[:, topi]                  [N, K]   (host, cheap O(N*K))
    Tg   = adj @ Sg                    [N, K]   (device matmul, dominant)
    Cg   = Sg^T @ Tg                   [K, K]   (device matmul)
    x_cg = Sg^T @ x                    [K, F]   (device matmul)

The device streams adj (16.8 MB/graph) once from HBM - that stream is the
roofline for this problem - while PE does the three matmuls in fp32r
(single-pass fp32 PE mode, 1 cyc/row at moving free dim >= 256). Cheap
O(N^2) host prep (row sums, one matvec for the GCN attention score, top-k)
and the scatter of the K nonzero columns/rows into the full-size outputs
are done host-side in numpy.

kernel(**inputs) takes FULL inputs and returns the FULL output tuple
(x_c, coarse, S, topi) exactly like the reference.
"""

import numpy as np

B, N, FDIM = 8, 2048, 256
K = 205          # int(N * 0.1) + 1
KP = 256         # K padded (zero cols); 256 unlocks the fp32r 1-cyc/row PE path
P = 128          # SBUF partitions
NT = N // P      # 16 row tiles
SW = 256         # adj strip width (columns per DMA)
NS = N // SW     # 8 strips

_CACHE = {}


def _build_program():
    import concourse.tile as tile
    from concourse import bacc, mybir

    nc = bacc.Bacc("TRN2", target_bir_lowering=False, debug=False, num_devices=B)

    f32 = mybir.dt.float32
    f32r = mybir.dt.float32r  # same bits as f32; fast single-pass PE mode

    # All inputs arrive host-pre-tiled to the exact SBUF layout so every
    # DMA is fully contiguous per partition (16-64 KB descriptors):
    # adj_t[s, p, i, c] = adj[i*128 + p, s*SW + c]
    # sg_t[p, i, k] = sg[i*128 + p, k];  x_t[p, i, f] = x[i*128 + p, f]
    adj = nc.dram_tensor("adj", [NS, P, NT, SW], f32r, kind="ExternalInput")
    sg = nc.dram_tensor("sg", [P, NT, KP], f32r, kind="ExternalInput")
    x = nc.dram_tensor("x", [P, NT, FDIM], f32r, kind="ExternalInput")
    cg = nc.dram_tensor("cg", [KP, KP], f32, kind="ExternalOutput")
    xc = nc.dram_tensor("xc", [KP, FDIM], f32, kind="ExternalOutput")

    with tile.TileContext(nc) as tc:
        with (
            tc.tile_pool(name="resident", bufs=1) as resident,
            tc.tile_pool(name="strips", bufs=4) as strips,
            tc.tile_pool(name="tgp", bufs=4) as tgp,
            tc.tile_pool(name="outp", bufs=1) as outp,
            tc.tile_pool(name="ps_t", bufs=3, space="PSUM") as ps_t,
            tc.tile_pool(name="ps_acc", bufs=1, space="PSUM") as ps_acc,
        ):
            # Resident operands: Sg [128, 16, 256] and x [128, 16, 256],
            # partition p holds row i*128+p of the logical [2048, *] matrix.
            # All input DMAs go on the sync HWDGE ring in priority order:
            # sg -> strip0 -> x(lo) -> strip1 -> x(hi) -> strips 2..7, so
            # the stream stays saturated and PE can start after ~4 MB.
            H = NT // 2
            sg_sb = resident.tile([P, NT, KP], f32r)
            x_sb = resident.tile([P, NT, FDIM], f32r)

            # PSUM accumulators held across the whole n loop.
            cg_ps0 = ps_acc.tile([P, KP], f32)
            cg_ps1 = ps_acc.tile([KP - P, KP], f32)
            xc_ps0 = ps_acc.tile([P, FDIM], f32)
            xc_ps1 = ps_acc.tile([KP - P, FDIM], f32)

            adj_r = adj.ap()  # [NS, 128, 16, SW], already SBUF-layout

            nc.sync.dma_start(out=sg_sb, in_=sg.ap())
            strip_tiles = []
            for s in range(NS):
                strip = strips.tile([P, NT, SW], f32r, tag="strip", name=f"strip{s}")
                strip_tiles.append(strip)
            nc.sync.dma_start(out=strip_tiles[0], in_=adj_r[0])
            nc.sync.dma_start(out=x_sb[:, 0:H, :], in_=x.ap()[:, 0:H, :])
            nc.sync.dma_start(out=strip_tiles[1], in_=adj_r[1])
            nc.sync.dma_start(out=x_sb[:, H:NT, :], in_=x.ap()[:, H:NT, :])
            for s in range(2, NS):
                nc.sync.dma_start(out=strip_tiles[s], in_=adj_r[s])

            for s in range(NS):
                strip = strip_tiles[s]
                for j in range(SW // P):
                    n = s * (SW // P) + j
                    # Tg[n-block] = sum_i adj[i-block, n-block].T @ Sg_i
                    # (adj is exactly symmetric, so adj[i,n] blocks are lhsT).
                    tg_ps = ps_t.tile([P, KP], f32, tag="tg_ps")
                    for i in range(NT):
                        nc.tensor.matmul(
                            tg_ps,
                            lhsT=strip[:, i, j * P : (j + 1) * P],
                            rhs=sg_sb[:, i, :],
                            start=(i == 0),
                            stop=(i == NT - 1),
                        )
                    tg_sb = tgp.tile([P, KP], f32r, tag="tg_sb")
                    nc.vector.tensor_copy(out=tg_sb, in_=tg_ps)

                    # Cg += Sg_n^T @ Tg_n ; x_cg += Sg_n^T @ x_n
                    nc.tensor.matmul(
                        cg_ps0, lhsT=sg_sb[:, n, 0:P], rhs=tg_sb,
                        start=(n == 0), stop=(n == NT - 1),
                    )
                    nc.tensor.matmul(
                        cg_ps1, lhsT=sg_sb[:, n, P:KP], rhs=tg_sb,
                        start=(n == 0), stop=(n == NT - 1),
                    )
                    nc.tensor.matmul(
                        xc_ps0, lhsT=sg_sb[:, n, 0:P], rhs=x_sb[:, n, :],
                        start=(n == 0), stop=(n == NT - 1),
                    )
                    nc.tensor.matmul(
                        xc_ps1, lhsT=sg_sb[:, n, P:KP], rhs=x_sb[:, n, :],
                        start=(n == 0), stop=(n == NT - 1),
                    )

            cg0 = outp.tile([P, KP], f32)
            nc.vector.tensor_copy(out=cg0, in_=cg_ps0)
            nc.scalar.dma_start(out=cg.ap()[0:P, :], in_=cg0)
            cg1 = outp.tile([KP - P, KP], f32)
            nc.vector.tensor_copy(out=cg1, in_=cg_ps1)
            nc.scalar.dma_start(out=cg.ap()[P:KP, :], in_=cg1)
            xc0 = outp.tile([P, FDIM], f32)
            nc.vector.tensor_copy(out=xc0, in_=xc_ps0)
            nc.scalar.dma_start(out=xc.ap()[0:P, :], in_=xc0)
            xc1 = outp.tile([KP - P, FDIM], f32)
            nc.vector.tensor_copy(out=xc1, in_=xc_ps1)
            nc.scalar.dma_start(out=xc.ap()[P:KP, :], in_=xc1)

    nc.compile()
    return nc


def _host_prep(x, adj, W, b):
    """O(N^2) prep: attention scores, top-k, and the K nonzero S columns."""
    f1 = np.float32
    xw = (x.reshape(B * N, FDIM) @ W).reshape(B, N).astype(f1)
    rs = adj.sum(axis=2, dtype=np.float32)            # [B, N]
    d = ((rs + f1(1.0)) ** f1(-0.5)).astype(f1)
    maskf = (rs > 0).astype(f1)
    v = (d * xw).astype(f1)
    Av = (np.matmul(adj, v[..., None])[..., 0] + v).astype(f1)
    z = (d * Av + b[0]).astype(f1)
    alpha = (f1(1.0) / (f1(1.0) + np.exp(-(z * z)))).astype(f1)

    order = np.argsort(-alpha, axis=1, kind="stable")
    topi = order[:, :K].astype(np.int32)

    sg_pad = np.zeros((B, N, KP), np.float32)
    dmask = (d * maskf).astype(f1)
    for g in range(B):
        ti = topi[g]
        cut = alpha[g, ti[K - 1]]
        cut_alpha = np.maximum((alpha[g] + f1(1e-7)) - cut, f1(0.0))
        w_vec = (d[g, ti] * cut_alpha[ti]).astype(f1)
        cols = adj[g][:, ti].copy()                   # [N, K] = A_hat columns
        cols[ti, np.arange(K)] += f1(1.0)             # + I on the diagonal
        sun = (dmask[g][:, None] * cols * w_vec[None, :]).astype(f1)
        s = sun.sum(axis=1, dtype=np.float32)
        q = (f1(1.0) / np.maximum(s, f1(1e-12))).astype(f1)
        sg_pad[g, :, :K] = sun * q[:, None]
    return sg_pad, topi


def _run_device(adj, sg_pad, x, trace=False, **kw):
    from concourse.bass_utils import run_bass_kernel_spmd

    if "nc" not in _CACHE:
        _CACHE["nc"] = _build_program()
    nc = _CACHE["nc"]
    in_maps = []
    for g in range(B):
        # adj_t[s, p, i, c] = adj[i*128 + p, s*SW + c]
        adj_t = np.ascontiguousarray(
            adj[g].reshape(NT, P, NS, SW).transpose(2, 1, 0, 3)
        )
        sg_t = np.ascontiguousarray(sg_pad[g].reshape(NT, P, KP).transpose(1, 0, 2))
        x_t = np.ascontiguousarray(x[g].reshape(NT, P, FDIM).transpose(1, 0, 2))
        in_maps.append({"adj": adj_t, "sg": sg_t, "x": x_t})
    return run_bass_kernel_spmd(nc, in_maps, list(range(B)), trace=trace, **kw)


def kernel(x, adj, W, b):
    x = np.ascontiguousarray(np.asarray(x, np.float32))
    adj = np.ascontiguousarray(np.asarray(adj, np.float32))
    W = np.asarray(W, np.float32)
    b = np.asarray(b, np.float32)

    sg_pad, topi = _host_prep(x, adj, W, b)
    res = _run_device(adj, sg_pad, x).results

    f1 = np.float32
    x_c = np.zeros((B, N, FDIM), np.float32)
    coarse = np.zeros((B, N, N), np.float32)
    S = np.zeros((B, N, N), np.float32)
    for g in range(B):
        ti = topi[g]
        cg_dev = res[g]["cg"][:K, :K]
        xc_dev = res[g]["xc"][:K, :]
        coarse[g][np.ix_(ti, ti)] = np.floor(cg_dev * f1(1e4)) / f1(1e4)
        x_c[g][ti, :] = xc_dev
        S[g][:, ti] = sg_pad[g, :, :K]
    return x_c, coarse, S, topi


# revision 19
# speedup vs baseline: 1.2803x; 1.0941x over previous
"""Trainium2 kernel for nn_CoarsenBlock (topk graph coarsening).

Strategy
--------
Per-graph math is independent -> data-parallel over B=8 graphs, one graph
per NeuronCore (8 cores).

The reference output S = norm_adj * cut_alpha has only K=205 nonzero
columns (the top-k attention nodes; verified: exactly K positive entries of
cut_alpha per graph, with >=1.6e-4 margin at the cut vs <=4e-7 float noise).
So the heavy O(N^3) dense einsums collapse to gathered forms:

    Sg   = S[:, topi]                  [N, K]   (host, cheap O(N*K))
    Tg   = adj @ Sg                    [N, K]   (device matmul, dominant)
    Cg   = Sg^T @ Tg                   [K, K]   (device matmul)
    x_cg = Sg^T @ x                    [K, F]   (device matmul)

The device streams adj (16.8 MB/graph) once from HBM - that stream is the
roofline for this problem - while PE does the three matmuls in fp32r
(single-pass fp32 PE mode, 1 cyc/row at moving free dim >= 256). Cheap
O(N^2) host prep (row sums, one matvec for the GCN attention score, top-k)
and the scatter of the K nonzero columns/rows into the full-size outputs
are done host-side in numpy.

kernel(**inputs) takes FULL inputs and returns the FULL output tuple
(x_c, coarse, S, topi) exactly like the reference.
"""

import numpy as np

B, N, FDIM = 8, 2048, 256
K = 205          # int(N * 0.1) + 1
KP = 256         # K padded (zero cols); 256 unlocks the fp32r 1-cyc/row PE path
P = 128          # SBUF partitions
NT = N // P      # 16 row tiles
SW = 256         # adj strip width (columns per DMA)
NS = N // SW     # 8 strips

_CACHE = {}


def _build_program():
    import concourse.tile as tile
    from concourse import bacc, mybir

    nc = bacc.Bacc("TRN2", target_bir_lowering=False, debug=False, num_devices=B)

    f32 = mybir.dt.float32
    f32r = mybir.dt.float32r  # same bits as f32; fast single-pass PE mode

    # All inputs arrive host-pre-tiled to the exact SBUF layout so every
    # DMA is fully contiguous per partition (16-64 KB descriptors):
    # adj_t[s, p, i, c] = adj[i*128 + p, s*SW + c]
    # sg_t[p, i, k] = sg[i*128 + p, k];  x_t[p, i, f] = x[i*128 + p, f]
    adj = nc.dram_tensor("adj", [NS, P, NT, SW], f32r, kind="ExternalInput")
    sg = nc.dram_tensor("sg", [P, NT, KP], f32r, kind="ExternalInput")
    x = nc.dram_tensor("x", [P, NT, FDIM], f32r, kind="ExternalInput")
    cg = nc.dram_tensor("cg", [KP, KP], f32, kind="ExternalOutput")
    xc = nc.dram_tensor("xc", [KP, FDIM], f32, kind="ExternalOutput")

    with tile.TileContext(nc) as tc:
        with (
            tc.tile_pool(name="resident", bufs=1) as resident,
            tc.tile_pool(name="strips", bufs=5) as strips,
            tc.tile_pool(name="tgp", bufs=4) as tgp,
            tc.tile_pool(name="outp", bufs=1) as outp,
            tc.tile_pool(name="ps_t", bufs=3, space="PSUM") as ps_t,
            tc.tile_pool(name="ps_acc", bufs=1, space="PSUM") as ps_acc,
        ):
            # Resident operands: Sg [128, 16, 256] and x [128, 16, 256],
            # partition p holds row i*128+p of the logical [2048, *] matrix.
            # All input DMAs go on the sync HWDGE ring in priority order:
            # sg -> strip0 -> x(lo) -> strip1 -> x(hi) -> strips 2..7, so
            # the stream stays saturated and PE can start after ~4 MB.
            H = NT // 2
            sg_sb = resident.tile([P, NT, KP], f32r)
            # x in two independent tiles so each half's D matmuls gate only
            # on their own DMA (whole-tile dep granularity).
            x_lo = resident.tile([P, H, FDIM], f32r)
            x_hi = resident.tile([P, H, FDIM], f32r)

            # PSUM accumulators held across the whole n loop.
            cg_ps0 = ps_acc.tile([P, KP], f32)
            cg_ps1 = ps_acc.tile([KP - P, KP], f32)
            xc_ps0 = ps_acc.tile([P, FDIM], f32)
            xc_ps1 = ps_acc.tile([KP - P, FDIM], f32)

            adj_r = adj.ap()  # [NS, 128, 16, SW], already SBUF-layout

            nc.sync.dma_start(out=sg_sb, in_=sg.ap())
            strip_tiles = []
            for s in range(NS):
                strip = strips.tile([P, NT, SW], f32r, tag="strip", name=f"strip{s}")
                strip_tiles.append(strip)
            nc.sync.dma_start(out=strip_tiles[0], in_=adj_r[0])
            nc.sync.dma_start(out=x_lo, in_=x.ap()[:, 0:H, :])
            nc.sync.dma_start(out=strip_tiles[1], in_=adj_r[1])
            nc.sync.dma_start(out=x_hi, in_=x.ap()[:, H:NT, :])
            for s in range(2, NS):
                nc.sync.dma_start(out=strip_tiles[s], in_=adj_r[s])

            for s in range(NS):
                strip = strip_tiles[s]
                for j in range(SW // P):
                    n = s * (SW // P) + j
                    # Tg[n-block] = sum_i adj[i-block, n-block].T @ Sg_i
                    # (adj is exactly symmetric, so adj[i,n] blocks are lhsT).
                    tg_ps = ps_t.tile([P, KP], f32, tag="tg_ps")
                    for i in range(NT):
                        nc.tensor.matmul(
                            tg_ps,
                            lhsT=strip[:, i, j * P : (j + 1) * P],
                            rhs=sg_sb[:, i, :],
                            start=(i == 0),
                            stop=(i == NT - 1),
                        )
                    tg_sb = tgp.tile([P, KP], f32r, tag="tg_sb")
                    nc.vector.tensor_copy(out=tg_sb, in_=tg_ps)

                    # Cg += Sg_n^T @ Tg_n ; x_cg += Sg_n^T @ x_n
                    nc.tensor.matmul(
                        cg_ps0, lhsT=sg_sb[:, n, 0:P], rhs=tg_sb,
                        start=(n == 0), stop=(n == NT - 1),
                    )
                    nc.tensor.matmul(
                        cg_ps1, lhsT=sg_sb[:, n, P:KP], rhs=tg_sb,
                        start=(n == 0), stop=(n == NT - 1),
                    )
                    x_n = x_lo[:, n, :] if n < H else x_hi[:, n - H, :]
                    nc.tensor.matmul(
                        xc_ps0, lhsT=sg_sb[:, n, 0:P], rhs=x_n,
                        start=(n == 0), stop=(n == NT - 1),
                    )
                    nc.tensor.matmul(
                        xc_ps1, lhsT=sg_sb[:, n, P:KP], rhs=x_n,
                        start=(n == 0), stop=(n == NT - 1),
                    )

            cg0 = outp.tile([P, KP], f32)
            nc.vector.tensor_copy(out=cg0, in_=cg_ps0)
            nc.scalar.dma_start(out=cg.ap()[0:P, :], in_=cg0)
            cg1 = outp.tile([KP - P, KP], f32)
            nc.vector.tensor_copy(out=cg1, in_=cg_ps1)
            nc.scalar.dma_start(out=cg.ap()[P:KP, :], in_=cg1)
            xc0 = outp.tile([P, FDIM], f32)
            nc.vector.tensor_copy(out=xc0, in_=xc_ps0)
            nc.scalar.dma_start(out=xc.ap()[0:P, :], in_=xc0)
            xc1 = outp.tile([KP - P, FDIM], f32)
            nc.vector.tensor_copy(out=xc1, in_=xc_ps1)
            nc.scalar.dma_start(out=xc.ap()[P:KP, :], in_=xc1)

    nc.compile()
    return nc


def _host_prep(x, adj, W, b):
    """O(N^2) prep: attention scores, top-k, and the K nonzero S columns."""
    f1 = np.float32
    xw = (x.reshape(B * N, FDIM) @ W).reshape(B, N).astype(f1)
    rs = adj.sum(axis=2, dtype=np.float32)            # [B, N]
    d = ((rs + f1(1.0)) ** f1(-0.5)).astype(f1)
    maskf = (rs > 0).astype(f1)
    v = (d * xw).astype(f1)
    Av = (np.matmul(adj, v[..., None])[..., 0] + v).astype(f1)
    z = (d * Av + b[0]).astype(f1)
    alpha = (f1(1.0) / (f1(1.0) + np.exp(-(z * z)))).astype(f1)

    order = np.argsort(-alpha, axis=1, kind="stable")
    topi = order[:, :K].astype(np.int32)

    sg_pad = np.zeros((B, N, KP), np.float32)
    dmask = (d * maskf).astype(f1)
    for g in range(B):
        ti = topi[g]
        cut = alpha[g, ti[K - 1]]
        cut_alpha = np.maximum((alpha[g] + f1(1e-7)) - cut, f1(0.0))
        w_vec = (d[g, ti] * cut_alpha[ti]).astype(f1)
        cols = adj[g][:, ti].copy()                   # [N, K] = A_hat columns
        cols[ti, np.arange(K)] += f1(1.0)             # + I on the diagonal
        sun = (dmask[g][:, None] * cols * w_vec[None, :]).astype(f1)
        s = sun.sum(axis=1, dtype=np.float32)
        q = (f1(1.0) / np.maximum(s, f1(1e-12))).astype(f1)
        sg_pad[g, :, :K] = sun * q[:, None]
    return sg_pad, topi


def _run_device(adj, sg_pad, x, trace=False, **kw):
    from concourse.bass_utils import run_bass_kernel_spmd

    if "nc" not in _CACHE:
        _CACHE["nc"] = _build_program()
    nc = _CACHE["nc"]
    in_maps = []
    for g in range(B):
        # adj_t[s, p, i, c] = adj[i*128 + p, s*SW + c]
        adj_t = np.ascontiguousarray(
            adj[g].reshape(NT, P, NS, SW).transpose(2, 1, 0, 3)
        )
        sg_t = np.ascontiguousarray(sg_pad[g].reshape(NT, P, KP).transpose(1, 0, 2))
        x_t = np.ascontiguousarray(x[g].reshape(NT, P, FDIM).transpose(1, 0, 2))
        in_maps.append({"adj": adj_t, "sg": sg_t, "x": x_t})
    return run_bass_kernel_spmd(nc, in_maps, list(range(B)), trace=trace, **kw)


def kernel(x, adj, W, b):
    x = np.ascontiguousarray(np.asarray(x, np.float32))
    adj = np.ascontiguousarray(np.asarray(adj, np.float32))
    W = np.asarray(W, np.float32)
    b = np.asarray(b, np.float32)

    sg_pad, topi = _host_prep(x, adj, W, b)
    res = _run_device(adj, sg_pad, x).results

    f1 = np.float32
    x_c = np.zeros((B, N, FDIM), np.float32)
    coarse = np.zeros((B, N, N), np.float32)
    S = np.zeros((B, N, N), np.float32)
    for g in range(B):
        ti = topi[g]
        cg_dev = res[g]["cg"][:K, :K]
        xc_dev = res[g]["xc"][:K, :]
        coarse[g][np.ix_(ti, ti)] = np.floor(cg_dev * f1(1e4)) / f1(1e4)
        x_c[g][ti, :] = xc_dev
        S[g][:, ti] = sg_pad[g, :, :K]
    return x_c, coarse, S, topi
